# revision 1
# baseline (speedup 1.0000x reference)
"""Eisner DP chart fill v3 — v2 + mid-DP sentence repack at k=64.

For k > 64 only span starts i < 128-k < 64 are valid, so half the
partitions idle. At the k=64 boundary DP state is repacked IN PLACE:
sentences 4..7 move into the upper 64 partitions of sentence-slot 0..3
(partition p holds i = p mod 64 of sentence s' + 4*(p>=64)), halving
the free size of every big op for k > 64. The region this overwrites
(sentences 0..3, i >= 64 — final after step 63, since their widths are
< 64) is deskewed to DRAM before the repack. Slide partition shifts
crossing the half boundary only contaminate invalid lanes.

See kernel2.py docstring for the core design (combined charts, fused
adds, max_index argmax with per-sentence 2^s scaling, DMA slides).
"""
import numpy as np
from contextlib import ExitStack

import concourse.bacc as bacc
import concourse.tile as tile
from concourse import mybir
from concourse.bass_types import AP
from concourse import bass_utils

N = 128
S = 8
SB = 4
KS = 64          # split step
NCORES = 8
DT = mybir.dt.float32
DI = mybir.dt.int32
DU = mybir.dt.uint16
NEGC = -9999.0
BON = 5.0

IN_SPECS = {
    "vpc": [S, N, 2 * N],
    "vpcT": [S, N, 2 * N],
    "shf1": [N, N],
    "skk00": [N, S, N],
    "skk11": [N, S, N],
    "bons": [N, S],
    "bonsb": [N, SB],
}
OUT_NAMES = ["sc00", "sc01", "sc10", "sc11", "bt00", "bt01", "bt11"]


def _host_consts():
    sh1 = np.zeros((N, N), np.float32)
    for p in range(N - 1):
        sh1[p + 1, p] = 1.0
    ii = np.arange(N, dtype=np.float32)[:, None, None]
    ss = np.arange(S, dtype=np.float32)[None, :, None]
    kk = np.arange(N, dtype=np.float32)[None, None, :]
    skk00 = (ii - ss * kk).astype(np.float32)
    # packed region (cols > KS): partition p = 64*half + i, slot sl<4 holds
    # sentence sl + 4*half
    ib = (np.arange(N) % 64).astype(np.float32)[:, None, None]
    sb = np.arange(S, dtype=np.float32)[None, :, None] % SB
    skk00b = (ib - sb * kk).astype(np.float32)
    skk00[:, :, KS + 1:] = skk00b[:, :, KS + 1:]
    skk11 = skk00 + 1.0
    bons = np.broadcast_to(
        (BON * (2.0 ** np.arange(S, dtype=np.float32)))[None, :], (N, S)
    ).copy()
    sfull = (np.arange(SB, dtype=np.float32)[None, :]
             + 4.0 * (np.arange(N)[:, None] >= 64))
    bonsb = (BON * (2.0 ** sfull)).astype(np.float32)
    return {"shf1": sh1, "skk00": skk00, "skk11": skk11, "bons": bons,
            "bonsb": bonsb}


def _pad_vinfo(v8):
    sc = (2.0 ** np.arange(S, dtype=np.float32))[:, None, None]
    vpc = np.zeros((S, N, 2 * N), np.float32)
    vpc[:, :, :N] = v8 * sc
    vpcT = np.zeros((S, N, 2 * N), np.float32)
    vpcT[:, :, :N] = v8.transpose(0, 2, 1) * sc
    return vpc, vpcT


def _emit(tc, outs, ins):
    nc = tc.nc
    ctx = ExitStack()
    P = ctx.enter_context(tc.tile_pool(name="pers", bufs=1))
    SC = ctx.enter_context(tc.tile_pool(name="scr", bufs=4))
    SM = ctx.enter_context(tc.tile_pool(name="sml", bufs=16))
    PS = ctx.enter_context(tc.tile_pool(name="psum", bufs=4, space="PSUM"))

    L = P.tile([N, 3, S, N], DT, tag="L")
    R = [P.tile([N, 3, S, N], DT, tag=f"R{b}", name=f"R{b}") for b in range(3)]
    S00 = P.tile([N, S, N], DT, tag="S00")
    IDX = P.tile([N, 3 * S, N], DU, tag="IDX")
    vL = P.tile([N, S, N], DT, tag="vL")
    vR = P.tile([N, S, N], DT, tag="vR")
    sh1 = P.tile([N, N], DT, tag="sh1")
    SKK0 = P.tile([N, S, N], DT, tag="SKK0")
    SKK1 = P.tile([N, S, N], DT, tag="SKK1")
    BONS = P.tile([N, S], DT, tag="BONS")
    BONSb = P.tile([N, SB], DT, tag="BONSb")

    nc.gpsimd.dma_start(BONS[:, :], ins["bons"])
    nc.gpsimd.dma_start(BONSb[:, :], ins["bonsb"])
    nc.gpsimd.dma_start(sh1[:, :], ins["shf1"])
    nc.gpsimd.dma_start(SKK0[:, :, :], ins["skk00"])
    nc.gpsimd.dma_start(SKK1[:, :, :], ins["skk11"])
    vhT = ins["vpcT"].tensor
    vh = ins["vpc"].tensor
    for s in range(S):
        nc.gpsimd.dma_start(
            vL[:, s, :], AP(vhT, s * 2 * N * N, [[2 * N + 1, N], [1, N]]))
        nc.gpsimd.dma_start(
            vR[:, s, :], AP(vh, s * 2 * N * N, [[2 * N + 1, N], [1, N]]))

    # init
    nc.vector.memset(L[:, :, :, :], NEGC)
    nc.gpsimd.memset(L[:, 0, :, 0], 0.0)
    nc.gpsimd.tensor_scalar_mul(L[:, 1, :, 0], BONS[:, :], -2000.0)
    nc.vector.memset(S00[:, :, :], 0.0)
    for b in range(3):
        nc.gpsimd.memset(R[b][:, :, :, :], 0.0)

    def step(k, Sb, BONSt):
        Rk = R[k % 3]
        X = SC.tile([N, 3, Sb, k], DT, tag="X")
        lo = min(2, k)
        # early bulk add [4:k] is 16B-aligned and reads only >=2-step-old
        # data; [2:4] waits on slide2 of step k-2; [0:2] on slide1 of k-1
        if k >= 5:
            nc.gpsimd.tensor_tensor(out=X[:, :, :, 4:k],
                                    in0=L[:, :, 0:Sb, 4:k],
                                    in1=Rk[:, :, 0:Sb, 4:k],
                                    op=mybir.AluOpType.add)
        if k >= 3:
            mi = min(4, k)
            nc.gpsimd.tensor_tensor(out=X[:, :, :, 2:mi],
                                    in0=L[:, :, 0:Sb, 2:mi],
                                    in1=Rk[:, :, 0:Sb, 2:mi],
                                    op=mybir.AluOpType.add)
        nc.gpsimd.tensor_tensor(out=X[:, :, :, 0:lo], in0=L[:, :, 0:Sb, 0:lo],
                                in1=Rk[:, :, 0:Sb, 0:lo],
                                op=mybir.AluOpType.add)
        M3 = SM.tile([N, 3, S], DT, tag="M3")
        ML = SM.tile([N, 3, S], DT, tag="ML")
        nc.vector.tensor_reduce(ML[:, :, 0:Sb], X[:, :, :, 0:lo],
                                axis=mybir.AxisListType.X,
                                op=mybir.AluOpType.max)
        if k >= 3:
            ME = SM.tile([N, 3, S], DT, tag="ME")
            nc.vector.tensor_reduce(ME[:, :, 0:Sb], X[:, :, :, 2:k],
                                    axis=mybir.AxisListType.X,
                                    op=mybir.AluOpType.max)
            nc.vector.tensor_tensor(out=M3[:, :, 0:Sb], in0=ME[:, :, 0:Sb],
                                    in1=ML[:, :, 0:Sb],
                                    op=mybir.AluOpType.max)
        else:
            nc.vector.tensor_copy(M3[:, :, 0:Sb], ML[:, :, 0:Sb])
        rb = M3[:, 0, 0:Sb]
        t0 = SM.tile([N, Sb], DT, tag="t0")
        nc.gpsimd.tensor_tensor(out=t0[:, :], in0=rb, in1=vL[:, 0:Sb, k],
                                op=mybir.AluOpType.add)
        nc.gpsimd.tensor_tensor(out=S00[:, 0:Sb, k], in0=t0[:, :],
                                in1=BONSt[:, :], op=mybir.AluOpType.add)
        nc.scalar.copy(Rk[:, 1, 0:Sb, 0], S00[:, 0:Sb, k])
        t1 = SM.tile([N, Sb], DT, tag="t1")
        nc.gpsimd.tensor_tensor(out=t1[:, :], in0=rb, in1=vR[:, 0:Sb, k],
                                op=mybir.AluOpType.add)
        nc.gpsimd.tensor_tensor(out=X[:, 2, :, k - 1], in0=t1[:, :],
                                in1=BONSt[:, :], op=mybir.AluOpType.add)
        nc.scalar.copy(L[:, 2, 0:Sb, k - 1], X[:, 2, :, k - 1])
        t2 = SM.tile([N, Sb], DT, tag="t2")
        nc.vector.tensor_tensor(out=t2[:, :], in0=Rk[:, 0, 0:Sb, 0],
                                in1=vL[:, 0:Sb, k], op=mybir.AluOpType.add)
        nc.vector.tensor_tensor(out=X[:, 1, :, 0], in0=t2[:, :],
                                in1=BONSt[:, :], op=mybir.AluOpType.add)
        nc.vector.tensor_tensor(out=L[:, 1, 0:Sb, k], in0=X[:, 1, :, 0],
                                in1=M3[:, 1, 0:Sb], op=mybir.AluOpType.max)
        nc.vector.tensor_tensor(out=L[:, 0, 0:Sb, k], in0=M3[:, 2, 0:Sb],
                                in1=X[:, 2, :, k - 1], op=mybir.AluOpType.max)
        # argmax; when Sb < 8 the query slots Sb..7 hold stale finite
        # junk that matches nothing (results land in unread IDX slots)
        q0, q1, q2 = M3[:, 0, :], L[:, 1, :, k], L[:, 0, :, k]
        nc.vector.max_index(IDX[:, 0:S, k], q0,
                            X[:, 0, :, :].rearrange("p s k -> p (s k)"))
        nc.vector.max_index(IDX[:, S:2 * S, k], q1,
                            X[:, 1, :, :].rearrange("p s k -> p (s k)"))
        nc.vector.max_index(IDX[:, 2 * S:3 * S, k], q2,
                            X[:, 2, :, :].rearrange("p s k -> p (s k)"))
        # slide1: stage fresh columns, one shift matmul, copy out
        if k <= N - 2:
            Rn = R[(k + 1) % 3]
            FC = SM.tile([N, 5 * Sb], DT, tag="FC")
            nc.scalar.copy(FC[:, 0:2 * Sb], L[:, 0:2, 0:Sb, k])
            nc.scalar.copy(FC[:, 2 * Sb:4 * Sb], Rk[:, 0:3:2, 0:Sb, 0])
            nc.scalar.copy(FC[:, 4 * Sb:5 * Sb], S00[:, 0:Sb, k])
            psF = PS.tile([N, 5 * Sb], DT, tag="psF")
            nc.tensor.matmul(psF[:, :], sh1[:, :], FC[:, :],
                             start=True, stop=True)
            nc.scalar.copy(Rn[:, 2, 0:Sb, 0], psF[:, 0:Sb])
            nc.scalar.copy(
                Rn[:, 0, 0:Sb, 0:2],
                psF[:, Sb:3 * Sb].rearrange("p (m s) -> p s m", m=2))
            nc.scalar.copy(Rn[:, 2, 0:Sb, 1], psF[:, 3 * Sb:4 * Sb])
            nc.scalar.copy(Rn[:, 1, 0:Sb, 1], psF[:, 4 * Sb:5 * Sb])
        # slide2 (three DMA queues)
        if k <= N - 3:
            R2n = R[(k + 2) % 3]
            nc.sync.dma_start(R2n[0:N - 2, 0, 0:Sb, 2:k + 2],
                              Rk[2:N, 0, 0:Sb, 0:k])
            nc.scalar.dma_start(R2n[0:N - 2, 1, 0:Sb, 2:k + 2],
                                Rk[2:N, 1, 0:Sb, 0:k])
            nc.gpsimd.dma_start(R2n[0:N - 2, 2, 0:Sb, 2:k + 2],
                                Rk[2:N, 2, 0:Sb, 0:k])

    for k in range(1, KS + 1):
        step(k, S, BONS)

    # ---- transition at k = KS -------------------------------------------
    # 1) pre-deskew the region the repack will overwrite: sentences 0..3,
    #    rows i in [64,128), widths < 64 (final after step 63)
    def pre_deskew(dram_ap, srct, wend, off=0):
        h = dram_ap.tensor
        for s in range(SB):
            nc.sync.dma_start(
                AP(h, s * N * 256 + 64 * 257 + off, [[257, 64], [1, wend]]),
                srct[64:N, s, 0:wend])

    pre_deskew(outs["sc00"], S00, KS + 1)
    pre_deskew(outs["sc01"], L[:, 1], KS + 1)
    pre_deskew(outs["sc10"], L[:, 2], KS, off=1)
    pre_deskew(outs["sc11"], L[:, 0], KS + 1)
    # 2) packed vL/vR for upper half: partition 64+i holds sentence 4+sp
    for sp in range(SB):
        s = sp + 4
        nc.gpsimd.dma_start(
            vL[64:N, sp, :],
            AP(vhT, s * 2 * N * N, [[2 * N + 1, 64], [1, N]]))
        nc.gpsimd.dma_start(
            vR[64:N, sp, :],
            AP(vh, s * 2 * N * N, [[2 * N + 1, 64], [1, N]]))
    # 3) repack: upper half of slots 0..3 <- lower half of slots 4..7
    bA, bB = (KS + 1) % 3, (KS + 2) % 3
    nc.scalar.dma_start(L[64:N, :, 0:SB, :], L[0:64, :, 4:S, :])
    nc.sync.dma_start(R[bA][64:N, :, 0:SB, :], R[bA][0:64, :, 4:S, :])
    nc.gpsimd.dma_start(R[bB][64:N, :, 0:SB, :], R[bB][0:64, :, 4:S, :])
    nc.scalar.dma_start(S00[64:N, 0:SB, :], S00[0:64, 4:S, :])

    for k in range(KS + 1, N):
        step(k, SB, BONSb)

    # ---- post-loop batched BT conversion (combined SKK tables)
    BT = {}
    for ci, (nm, skk) in enumerate((("bt00", SKK0), ("bt01", SKK0),
                                    ("bt11", SKK1))):
        idxf = SC.tile([N, S, N], DT, tag=f"idxf{ci}")
        nc.vector.tensor_copy(idxf[:, :, :], IDX[:, ci * S:(ci + 1) * S, :])
        bt = P.tile([N, S, N], DI, tag=f"BT{ci}")
        nc.vector.tensor_tensor(out=bt[:, :, :], in0=idxf[:, :, :],
                                in1=skk[:, :, :], op=mybir.AluOpType.add)
        BT[nm] = bt

    # ---- final deskew ----------------------------------------------------
    def deskew_old(dram_ap, srct, wend, off=0, lowhalf_s03=False):
        h = dram_ap.tensor
        for s in range(S):
            rows = 64 if (lowhalf_s03 and s < SB) else N
            nc.sync.dma_start(
                AP(h, s * N * 256 + off, [[257, rows], [1, wend]]),
                srct[0:rows, s, 0:wend])

    def deskew_packed(dram_ap, srct, w0, off=0):
        h = dram_ap.tensor
        nw = N - w0
        for sp in range(SB):
            for half in range(2):
                s = sp + 4 * half
                p0 = 64 * half
                nc.sync.dma_start(
                    AP(h, s * N * 256 + w0 + off, [[257, 63], [1, nw]]),
                    srct[p0:p0 + 63, sp, w0:w0 + nw])

    # score charts: lower-half rows for s<4 (upper rows were pre-deskewed)
    deskew_old(outs["sc00"], S00, KS + 1, lowhalf_s03=True)
    deskew_packed(outs["sc00"], S00, KS + 1)
    deskew_old(outs["sc01"], L[:, 1], KS + 1, lowhalf_s03=True)
    deskew_packed(outs["sc01"], L[:, 1], KS + 1)
    deskew_old(outs["sc10"], L[:, 2], KS, off=1, lowhalf_s03=True)
    deskew_packed(outs["sc10"], L[:, 2], KS, off=1)
    deskew_old(outs["sc11"], L[:, 0], KS + 1, lowhalf_s03=True)
    deskew_packed(outs["sc11"], L[:, 0], KS + 1)
    # bt charts: IDX survived intact, so old region covers all rows
    for nm in ("bt00", "bt01", "bt11"):
        deskew_old(outs[nm], BT[nm], KS + 1)
        deskew_packed(outs[nm], BT[nm], KS + 1)
    ctx.close()


_NC_CACHE = None


def _build():
    global _NC_CACHE
    if _NC_CACHE is not None:
        return _NC_CACHE
    nc = bacc.Bacc("TRN2", target_bir_lowering=False, debug=False,
                   enable_asserts=False, num_devices=NCORES)
    ins = {nm: nc.dram_tensor(nm, sh, DT, kind="ExternalInput").ap()
           for nm, sh in IN_SPECS.items()}
    outs = {}
    for nm in OUT_NAMES:
        dt = DT if nm.startswith("sc") else DI
        outs[nm] = nc.dram_tensor(nm, [S, N, 2 * N], dt,
                                  kind="ExternalOutput").ap()
    with tile.TileContext(nc) as tc:
        _emit(tc, outs, ins)
    nc.compile()
    _NC_CACHE = nc
    return nc


_LAST_EXEC_NS = None


def kernel(b_vinfo_mtx, b_buffer_size, _trace=False):
    global _LAST_EXEC_NS
    v = np.ascontiguousarray(np.asarray(b_vinfo_mtx, dtype=np.float32))
    assert v.shape == (NCORES * S, N, N)
    consts = _host_consts()
    in_maps = []
    for c in range(NCORES):
        vpc, vpcT = _pad_vinfo(v[c * S:(c + 1) * S])
        in_maps.append({"vpc": vpc, "vpcT": vpcT, **consts})

    nc = _build()
    res = bass_utils.run_bass_kernel_spmd(
        nc, in_maps, core_ids=list(range(NCORES)), trace=_trace)
    _LAST_EXEC_NS = res.exec_time_ns

    scores = np.full((NCORES * S, N, N, 2, 2), NEGC, np.float32)
    bt = np.zeros((NCORES * S, N, N, 2, 2), np.int32)
    names = {"sc00": (0, 0), "sc01": (0, 1), "sc10": (1, 0), "sc11": (1, 1)}
    tri = np.tril_indices(N, k=-1)
    dg = np.arange(N)
    for c in range(NCORES):
        r = res.results[c]
        dsc = (2.0 ** -np.arange(S, dtype=np.float32))[:, None, None]
        for nm, (a, b) in names.items():
            sc = r[nm].reshape(S, N, 2 * N)[:, :, :N] * dsc
            sc = sc.astype(np.float32)
            sc[:, tri[0], tri[1]] = NEGC
            sc[:, dg, dg] = 0.0
            scores[c * S:(c + 1) * S, :, :, a, b] = sc
        for nm, (a, b) in (("bt00", (0, 0)), ("bt01", (0, 1)),
                           ("bt00", (1, 0)), ("bt11", (1, 1))):
            bb = r[nm].reshape(S, N, 2 * N)[:, :, :N].copy()
            bb[:, tri[0], tri[1]] = 0
            bb[:, dg, dg] = 0
            bt[c * S:(c + 1) * S, :, :, a, b] = bb
    return scores, bt



# revision 2
# speedup vs baseline: 1.0179x; 1.0179x over previous
"""Eisner DP chart fill v4 — v3 + host-prebiased arc scores + DMA offload.

v3 core (mid-DP repack at k=64) with three surgical changes:
  * vpc/vpcT are host-prebiased: (vinfo + 5) * 2^s, collapsing the per-step
    t0/t1/t2 double-adds into single fused adds (shorter M3 -> column chain,
    three fewer Pool ops per step).
  * slide2 combo-2 and the init/const loads moved off the GpSimd engine
    (gpsimd.dma_start executes as DIRECT2D *on* Pool, ~1.1us/step stalling
    the mid-step adds) onto the sync HWDGE queue.
  * repack R[bB] move likewise on sync.

For k > 64 only span starts i < 128-k < 64 are valid, so half the
partitions idle. At the k=64 boundary DP state is repacked IN PLACE:
sentences 4..7 move into the upper 64 partitions of sentence-slot 0..3
(partition p holds i = p mod 64 of sentence s' + 4*(p>=64)), halving
the free size of every big op for k > 64. The region this overwrites
(sentences 0..3, i >= 64 — final after step 63, since their widths are
< 64) is deskewed to DRAM before the repack. Slide partition shifts
crossing the half boundary only contaminate invalid lanes.

See kernel2.py docstring for the core design (combined charts, fused
adds, max_index argmax with per-sentence 2^s scaling, DMA slides).
"""
import numpy as np
from contextlib import ExitStack

import concourse.bacc as bacc
import concourse.tile as tile
from concourse import mybir
from concourse.bass_types import AP
from concourse import bass_utils

N = 128
S = 8
SB = 4
KS = 64          # split step
NCORES = 8
DT = mybir.dt.float32
DI = mybir.dt.int32
DU = mybir.dt.uint16
NEGC = -9999.0
BON = 5.0

IN_SPECS = {
    "vpc": [S, N, 2 * N],
    "vpcT": [S, N, 2 * N],
    "shf1": [N, N],
    "skk00": [N, S, N],
    "skk11": [N, S, N],
    "bons": [N, S],
    "bonsb": [N, SB],
}
OUT_NAMES = ["sc00", "sc01", "sc10", "sc11", "bt00", "bt01", "bt11"]


def _host_consts():
    sh1 = np.zeros((N, N), np.float32)
    for p in range(N - 1):
        sh1[p + 1, p] = 1.0
    ii = np.arange(N, dtype=np.float32)[:, None, None]
    ss = np.arange(S, dtype=np.float32)[None, :, None]
    kk = np.arange(N, dtype=np.float32)[None, None, :]
    skk00 = (ii - ss * kk).astype(np.float32)
    # packed region (cols > KS): partition p = 64*half + i, slot sl<4 holds
    # sentence sl + 4*half
    ib = (np.arange(N) % 64).astype(np.float32)[:, None, None]
    sb = np.arange(S, dtype=np.float32)[None, :, None] % SB
    skk00b = (ib - sb * kk).astype(np.float32)
    skk00[:, :, KS + 1:] = skk00b[:, :, KS + 1:]
    skk11 = skk00 + 1.0
    bons = np.broadcast_to(
        (BON * (2.0 ** np.arange(S, dtype=np.float32)))[None, :], (N, S)
    ).copy()
    sfull = (np.arange(SB, dtype=np.float32)[None, :]
             + 4.0 * (np.arange(N)[:, None] >= 64))
    bonsb = (BON * (2.0 ** sfull)).astype(np.float32)
    return {"shf1": sh1, "skk00": skk00, "skk11": skk11, "bons": bons,
            "bonsb": bonsb}


def _pad_vinfo(v8):
    sc = (2.0 ** np.arange(S, dtype=np.float32))[:, None, None]
    vpc = np.zeros((S, N, 2 * N), np.float32)
    vpc[:, :, :N] = (v8 + BON) * sc
    vpcT = np.zeros((S, N, 2 * N), np.float32)
    vpcT[:, :, :N] = (v8.transpose(0, 2, 1) + BON) * sc
    return vpc, vpcT


def _emit(tc, outs, ins):
    nc = tc.nc
    ctx = ExitStack()
    P = ctx.enter_context(tc.tile_pool(name="pers", bufs=1))
    SC = ctx.enter_context(tc.tile_pool(name="scr", bufs=4))
    SM = ctx.enter_context(tc.tile_pool(name="sml", bufs=16))
    PS = ctx.enter_context(tc.tile_pool(name="psum", bufs=4, space="PSUM"))

    L = P.tile([N, 3, S, N], DT, tag="L")
    R = [P.tile([N, 3, S, N], DT, tag=f"R{b}", name=f"R{b}") for b in range(3)]
    S00 = P.tile([N, S, N], DT, tag="S00")
    IDX = P.tile([N, 3 * S, N], DU, tag="IDX")
    vL = P.tile([N, S, N], DT, tag="vL")
    vR = P.tile([N, S, N], DT, tag="vR")
    sh1 = P.tile([N, N], DT, tag="sh1")
    SKK0 = P.tile([N, S, N], DT, tag="SKK0")
    SKK1 = P.tile([N, S, N], DT, tag="SKK1")
    BONS = P.tile([N, S], DT, tag="BONS")
    BONSb = P.tile([N, SB], DT, tag="BONSb")

    nc.sync.dma_start(BONS[:, :], ins["bons"])
    nc.sync.dma_start(BONSb[:, :], ins["bonsb"])
    nc.sync.dma_start(sh1[:, :], ins["shf1"])
    nc.sync.dma_start(SKK0[:, :, :], ins["skk00"])
    nc.sync.dma_start(SKK1[:, :, :], ins["skk11"])
    vhT = ins["vpcT"].tensor
    vh = ins["vpc"].tensor
    for s in range(S):
        nc.sync.dma_start(
            vL[:, s, :], AP(vhT, s * 2 * N * N, [[2 * N + 1, N], [1, N]]))
        nc.sync.dma_start(
            vR[:, s, :], AP(vh, s * 2 * N * N, [[2 * N + 1, N], [1, N]]))

    # init
    nc.vector.memset(L[:, :, :, :], NEGC)
    nc.gpsimd.memset(L[:, 0, :, 0], 0.0)
    nc.gpsimd.tensor_scalar_mul(L[:, 1, :, 0], BONS[:, :], -2000.0)
    nc.vector.memset(S00[:, :, :], 0.0)
    for b in range(3):
        nc.gpsimd.memset(R[b][:, :, :, :], 0.0)

    def step(k, Sb, BONSt):
        Rk = R[k % 3]
        X = SC.tile([N, 3, Sb, k], DT, tag="X")
        lo = min(2, k)
        # early bulk add [4:k] is 16B-aligned and reads only >=2-step-old
        # data; [2:4] waits on slide2 of step k-2; [0:2] on slide1 of k-1
        if k >= 5:
            nc.gpsimd.tensor_tensor(out=X[:, :, :, 4:k],
                                    in0=L[:, :, 0:Sb, 4:k],
                                    in1=Rk[:, :, 0:Sb, 4:k],
                                    op=mybir.AluOpType.add)
        if k >= 3:
            mi = min(4, k)
            nc.gpsimd.tensor_tensor(out=X[:, :, :, 2:mi],
                                    in0=L[:, :, 0:Sb, 2:mi],
                                    in1=Rk[:, :, 0:Sb, 2:mi],
                                    op=mybir.AluOpType.add)
        nc.gpsimd.tensor_tensor(out=X[:, :, :, 0:lo], in0=L[:, :, 0:Sb, 0:lo],
                                in1=Rk[:, :, 0:Sb, 0:lo],
                                op=mybir.AluOpType.add)
        M3 = SM.tile([N, 3, S], DT, tag="M3")
        ML = SM.tile([N, 3, S], DT, tag="ML")
        nc.vector.tensor_reduce(ML[:, :, 0:Sb], X[:, :, :, 0:lo],
                                axis=mybir.AxisListType.X,
                                op=mybir.AluOpType.max)
        if k >= 3:
            ME = SM.tile([N, 3, S], DT, tag="ME")
            nc.vector.tensor_reduce(ME[:, :, 0:Sb], X[:, :, :, 2:k],
                                    axis=mybir.AxisListType.X,
                                    op=mybir.AluOpType.max)
            nc.vector.tensor_tensor(out=M3[:, :, 0:Sb], in0=ME[:, :, 0:Sb],
                                    in1=ML[:, :, 0:Sb],
                                    op=mybir.AluOpType.max)
        else:
            nc.vector.tensor_copy(M3[:, :, 0:Sb], ML[:, :, 0:Sb])
        rb = M3[:, 0, 0:Sb]
        nc.gpsimd.tensor_tensor(out=S00[:, 0:Sb, k], in0=rb,
                                in1=vL[:, 0:Sb, k], op=mybir.AluOpType.add)
        nc.scalar.copy(Rk[:, 1, 0:Sb, 0], S00[:, 0:Sb, k])
        nc.gpsimd.tensor_tensor(out=X[:, 2, :, k - 1], in0=rb,
                                in1=vR[:, 0:Sb, k], op=mybir.AluOpType.add)
        nc.scalar.copy(L[:, 2, 0:Sb, k - 1], X[:, 2, :, k - 1])
        nc.vector.tensor_tensor(out=X[:, 1, :, 0], in0=Rk[:, 0, 0:Sb, 0],
                                in1=vL[:, 0:Sb, k], op=mybir.AluOpType.add)
        nc.vector.tensor_tensor(out=L[:, 1, 0:Sb, k], in0=X[:, 1, :, 0],
                                in1=M3[:, 1, 0:Sb], op=mybir.AluOpType.max)
        nc.vector.tensor_tensor(out=L[:, 0, 0:Sb, k], in0=M3[:, 2, 0:Sb],
                                in1=X[:, 2, :, k - 1], op=mybir.AluOpType.max)
        # argmax; when Sb < 8 the query slots Sb..7 hold stale finite
        # junk that matches nothing (results land in unread IDX slots)
        q0, q1, q2 = M3[:, 0, :], L[:, 1, :, k], L[:, 0, :, k]
        nc.vector.max_index(IDX[:, 0:S, k], q0,
                            X[:, 0, :, :].rearrange("p s k -> p (s k)"))
        nc.vector.max_index(IDX[:, S:2 * S, k], q1,
                            X[:, 1, :, :].rearrange("p s k -> p (s k)"))
        nc.vector.max_index(IDX[:, 2 * S:3 * S, k], q2,
                            X[:, 2, :, :].rearrange("p s k -> p (s k)"))
        # slide1: stage fresh columns, one shift matmul, copy out
        if k <= N - 2:
            Rn = R[(k + 1) % 3]
            FC = SM.tile([N, 5 * Sb], DT, tag="FC")
            nc.scalar.copy(FC[:, 0:2 * Sb], L[:, 0:2, 0:Sb, k])
            nc.scalar.copy(FC[:, 2 * Sb:4 * Sb], Rk[:, 0:3:2, 0:Sb, 0])
            nc.scalar.copy(FC[:, 4 * Sb:5 * Sb], S00[:, 0:Sb, k])
            psF = PS.tile([N, 5 * Sb], DT, tag="psF")
            nc.tensor.matmul(psF[:, :], sh1[:, :], FC[:, :],
                             start=True, stop=True)
            nc.scalar.copy(Rn[:, 2, 0:Sb, 0], psF[:, 0:Sb])
            nc.scalar.copy(
                Rn[:, 0, 0:Sb, 0:2],
                psF[:, Sb:3 * Sb].rearrange("p (m s) -> p s m", m=2))
            nc.scalar.copy(Rn[:, 2, 0:Sb, 1], psF[:, 3 * Sb:4 * Sb])
            nc.scalar.copy(Rn[:, 1, 0:Sb, 1], psF[:, 4 * Sb:5 * Sb])
        # slide2 (three DMA queues)
        if k <= N - 3:
            R2n = R[(k + 2) % 3]
            nc.sync.dma_start(R2n[0:N - 2, 0, 0:Sb, 2:k + 2],
                              Rk[2:N, 0, 0:Sb, 0:k])
            nc.scalar.dma_start(R2n[0:N - 2, 1, 0:Sb, 2:k + 2],
                                Rk[2:N, 1, 0:Sb, 0:k])
            nc.sync.dma_start(R2n[0:N - 2, 2, 0:Sb, 2:k + 2],
                                Rk[2:N, 2, 0:Sb, 0:k])

    for k in range(1, KS + 1):
        step(k, S, BONS)

    # ---- transition at k = KS -------------------------------------------
    # 1) pre-deskew the region the repack will overwrite: sentences 0..3,
    #    rows i in [64,128), widths < 64 (final after step 63)
    def pre_deskew(dram_ap, srct, wend, off=0):
        h = dram_ap.tensor
        for s in range(SB):
            nc.sync.dma_start(
                AP(h, s * N * 256 + 64 * 257 + off, [[257, 64], [1, wend]]),
                srct[64:N, s, 0:wend])

    pre_deskew(outs["sc00"], S00, KS + 1)
    pre_deskew(outs["sc01"], L[:, 1], KS + 1)
    pre_deskew(outs["sc10"], L[:, 2], KS, off=1)
    pre_deskew(outs["sc11"], L[:, 0], KS + 1)
    # 2) packed vL/vR for upper half: partition 64+i holds sentence 4+sp
    for sp in range(SB):
        s = sp + 4
        nc.gpsimd.dma_start(
            vL[64:N, sp, :],
            AP(vhT, s * 2 * N * N, [[2 * N + 1, 64], [1, N]]))
        nc.gpsimd.dma_start(
            vR[64:N, sp, :],
            AP(vh, s * 2 * N * N, [[2 * N + 1, 64], [1, N]]))
    # 3) repack: upper half of slots 0..3 <- lower half of slots 4..7
    bA, bB = (KS + 1) % 3, (KS + 2) % 3
    nc.scalar.dma_start(L[64:N, :, 0:SB, :], L[0:64, :, 4:S, :])
    nc.sync.dma_start(R[bA][64:N, :, 0:SB, :], R[bA][0:64, :, 4:S, :])
    nc.sync.dma_start(R[bB][64:N, :, 0:SB, :], R[bB][0:64, :, 4:S, :])
    nc.scalar.dma_start(S00[64:N, 0:SB, :], S00[0:64, 4:S, :])

    for k in range(KS + 1, N):
        step(k, SB, BONSb)

    # ---- post-loop batched BT conversion (combined SKK tables)
    BT = {}
    for ci, (nm, skk) in enumerate((("bt00", SKK0), ("bt01", SKK0),
                                    ("bt11", SKK1))):
        idxf = SC.tile([N, S, N], DT, tag=f"idxf{ci}")
        nc.vector.tensor_copy(idxf[:, :, :], IDX[:, ci * S:(ci + 1) * S, :])
        bt = P.tile([N, S, N], DI, tag=f"BT{ci}")
        nc.vector.tensor_tensor(out=bt[:, :, :], in0=idxf[:, :, :],
                                in1=skk[:, :, :], op=mybir.AluOpType.add)
        BT[nm] = bt

    # ---- final deskew ----------------------------------------------------
    def deskew_old(dram_ap, srct, wend, off=0, lowhalf_s03=False):
        h = dram_ap.tensor
        for s in range(S):
            rows = 64 if (lowhalf_s03 and s < SB) else N
            nc.sync.dma_start(
                AP(h, s * N * 256 + off, [[257, rows], [1, wend]]),
                srct[0:rows, s, 0:wend])

    def deskew_packed(dram_ap, srct, w0, off=0):
        h = dram_ap.tensor
        nw = N - w0
        for sp in range(SB):
            for half in range(2):
                s = sp + 4 * half
                p0 = 64 * half
                nc.sync.dma_start(
                    AP(h, s * N * 256 + w0 + off, [[257, 63], [1, nw]]),
                    srct[p0:p0 + 63, sp, w0:w0 + nw])

    # score charts: lower-half rows for s<4 (upper rows were pre-deskewed)
    deskew_old(outs["sc00"], S00, KS + 1, lowhalf_s03=True)
    deskew_packed(outs["sc00"], S00, KS + 1)
    deskew_old(outs["sc01"], L[:, 1], KS + 1, lowhalf_s03=True)
    deskew_packed(outs["sc01"], L[:, 1], KS + 1)
    deskew_old(outs["sc10"], L[:, 2], KS, off=1, lowhalf_s03=True)
    deskew_packed(outs["sc10"], L[:, 2], KS, off=1)
    deskew_old(outs["sc11"], L[:, 0], KS + 1, lowhalf_s03=True)
    deskew_packed(outs["sc11"], L[:, 0], KS + 1)
    # bt charts: IDX survived intact, so old region covers all rows
    for nm in ("bt00", "bt01", "bt11"):
        deskew_old(outs[nm], BT[nm], KS + 1)
        deskew_packed(outs[nm], BT[nm], KS + 1)
    ctx.close()


_NC_CACHE = None


def _build():
    global _NC_CACHE
    if _NC_CACHE is not None:
        return _NC_CACHE
    nc = bacc.Bacc("TRN2", target_bir_lowering=False, debug=False,
                   enable_asserts=False, num_devices=NCORES)
    ins = {nm: nc.dram_tensor(nm, sh, DT, kind="ExternalInput").ap()
           for nm, sh in IN_SPECS.items()}
    outs = {}
    for nm in OUT_NAMES:
        dt = DT if nm.startswith("sc") else DI
        outs[nm] = nc.dram_tensor(nm, [S, N, 2 * N], dt,
                                  kind="ExternalOutput").ap()
    with tile.TileContext(nc) as tc:
        _emit(tc, outs, ins)
    nc.compile()
    _NC_CACHE = nc
    return nc


_LAST_EXEC_NS = None


def kernel(b_vinfo_mtx, b_buffer_size, _trace=False):
    global _LAST_EXEC_NS
    v = np.ascontiguousarray(np.asarray(b_vinfo_mtx, dtype=np.float32))
    assert v.shape == (NCORES * S, N, N)
    consts = _host_consts()
    in_maps = []
    for c in range(NCORES):
        vpc, vpcT = _pad_vinfo(v[c * S:(c + 1) * S])
        in_maps.append({"vpc": vpc, "vpcT": vpcT, **consts})

    nc = _build()
    res = bass_utils.run_bass_kernel_spmd(
        nc, in_maps, core_ids=list(range(NCORES)), trace=_trace)
    _LAST_EXEC_NS = res.exec_time_ns

    scores = np.full((NCORES * S, N, N, 2, 2), NEGC, np.float32)
    bt = np.zeros((NCORES * S, N, N, 2, 2), np.int32)
    names = {"sc00": (0, 0), "sc01": (0, 1), "sc10": (1, 0), "sc11": (1, 1)}
    tri = np.tril_indices(N, k=-1)
    dg = np.arange(N)
    for c in range(NCORES):
        r = res.results[c]
        dsc = (2.0 ** -np.arange(S, dtype=np.float32))[:, None, None]
        for nm, (a, b) in names.items():
            sc = r[nm].reshape(S, N, 2 * N)[:, :, :N] * dsc
            sc = sc.astype(np.float32)
            sc[:, tri[0], tri[1]] = NEGC
            sc[:, dg, dg] = 0.0
            scores[c * S:(c + 1) * S, :, :, a, b] = sc
        for nm, (a, b) in (("bt00", (0, 0)), ("bt01", (0, 1)),
                           ("bt00", (1, 0)), ("bt11", (1, 1))):
            bb = r[nm].reshape(S, N, 2 * N)[:, :, :N].copy()
            bb[:, tri[0], tri[1]] = 0
            bb[:, dg, dg] = 0
            bt[c * S:(c + 1) * S, :, :, a, b] = bb
    return scores, bt

